# revision 16
# baseline (speedup 1.0000x reference)
"""Mixtral MoE (T=4096, H=1024, I=2048, E=8, top-2) on 8 TRN2 NeuronCores.

Expert-parallel: one expert per core.  Each core:
  - computes the router (f32, exact top-2-of-8 via max/is_equal algebra),
  - runs its expert's SwiGLU FFN in bf16 over token chunks,
  - scales by the per-token combine weight,
  - joins partial outputs across cores with a bf16 ReduceScatter per
    1024-token block (overlapped with compute of later blocks).

Host side only reshapes/transposes inputs (layout prep) and concatenates
the per-core ReduceScatter shards back into the full [1,4096,1024] output.
"""

import numpy as np

import concourse.bass as bass
import concourse.mybir as mybir
import concourse.tile as tile
from concourse import mybir as _mybir
from concourse.bass_utils import run_bass_kernel_spmd
from concourse.masks import make_identity
from concourse.vector_clock import ScopedClock

F32 = mybir.dt.float32
BF16 = mybir.dt.bfloat16
AF = mybir.ActivationFunctionType
ALU = mybir.AluOpType
AX = mybir.AxisListType

T, H, I, E = 4096, 1024, 2048, 8
NCORES = 8
P = 128
KT = H // P            # 8  k-tiles over H (contraction for h1/h3, output tiles for down)
IT = I // P            # 16 i-tiles over intermediate
CHUNK = 512            # tokens per FFN chunk (one PSUM bank at f32)
NCHUNK = T // CHUNK    # 8
TT = CHUNK // P        # 4  token-tiles per chunk (router granularity)
RS_TOK = 1024          # tokens per ReduceScatter block
NRS = T // RS_TOK      # 4
CPR = RS_TOK // CHUNK  # chunks per RS block = 2


# ---------------------------------------------------------------- tile patch
def _apply_tile_drain_patch():
    """This walrus build rejects >1 sync wait on a CTRL (Drain) instruction.
    Spread Tile's tail-drain waits across a chain of single-wait Drains."""

    def _patched(self, tick_clock, wait_clock):
        nc = self.nc
        drain_inst = nc.sync.drain()
        wait_clock.add_sem_waits(
            drain_inst.ins, ScopedClock({None: tick_clock.global_clock})
        )
        si = drain_inst.ins.sync_info
        waits = list(si.on_wait) if si is not None else []
        if len(waits) > 1:
            drain_inst.ins.sync_info = _mybir.SyncInfo(
                on_wait=waits[:1], on_update=list(si.on_update)
            )
            for w in waits[1:]:
                d = nc.sync.drain()
                d.ins.sync_info = _mybir.SyncInfo(on_wait=[w], on_update=[])
        nc.all_engine_barrier()
        assert self.sems is not None
        popped = nc._tile_sem_poison_stack.pop()
        assert popped is self._sem_poison
        nc.clear_and_free_semaphores(list(self.sems.allocated().values()))
        nc.all_engine_barrier()

    tile.TileContext._drain_and_barrier = _patched


_apply_tile_drain_patch()


def _hoist_extra_waits(nc, max_waits=1):
    """This walrus build allows only one sync-wait command per instruction.
    Hoist extra on_wait conditions onto same-engine NoOps inserted just
    before the over-limit instruction (sequencers execute in order, so the
    waits still complete before the instruction issues)."""
    for bb in nc.main_func.blocks:
        lst = bb.instructions
        i = 0
        while i < len(lst):
            ins = lst[i]
            si = getattr(ins, "sync_info", None)
            if si is not None and len(si.on_wait) > max_waits:
                waits = list(si.on_wait)
                ins.sync_info = _mybir.SyncInfo(
                    on_wait=waits[:max_waits], on_update=list(si.on_update)
                )
                for w in reversed(waits[max_waits:]):
                    nop = _mybir.InstNoOp(
                        name=nc.get_next_instruction_name(), ins=[], outs=[]
                    )
                    nop.engine = ins.engine
                    nop.sync_info = _mybir.SyncInfo(on_wait=[w], on_update=[])
                    lst.insert(i, nop)
                    i += 1
            i += 1


# ---------------------------------------------------------------- bass kernel
def build_nc(debug=False):
    nc = bass.Bass()

    xT_d = nc.declare_dram_parameter("xT", [H, T], F32, isOutput=False)
    wgT_d = nc.declare_dram_parameter("wgT", [H, E], F32, isOutput=False)
    w1T_d = nc.declare_dram_parameter("w1T", [H, I], F32, isOutput=False)
    w3T_d = nc.declare_dram_parameter("w3T", [H, I], F32, isOutput=False)
    w2T_d = nc.declare_dram_parameter("w2T", [I, H], F32, isOutput=False)
    out_d = nc.declare_dram_parameter("out", [NRS, P, RS_TOK], F32, isOutput=True)
    if debug:
        dbg_lch = nc.declare_dram_parameter("dbg_lch", [P, TT * E], F32, isOutput=True)
        dbg_wc = nc.declare_dram_parameter("dbg_wc", [P, TT], F32, isOutput=True)
        dbg_wbc = nc.declare_dram_parameter("dbg_wbc", [P, CHUNK], F32, isOutput=True)
        dbg_z = nc.declare_dram_parameter("dbg_z", [P, CHUNK], F32, isOutput=True)
        dbg_part = nc.declare_dram_parameter("dbg_part", [H, RS_TOK], F32, isOutput=True)

    with tile.TileContext(nc) as tc:
        with (
            tc.tile_pool(name="wpool", bufs=1) as wpool,
            tc.tile_pool(name="wload", bufs=2) as wload,
            tc.tile_pool(name="xf", bufs=1) as xf_pool,
            tc.tile_pool(name="xb", bufs=2) as xb_pool,
            tc.tile_pool(name="zp", bufs=1) as z_pool,
            tc.tile_pool(name="small", bufs=3) as small,
            tc.tile_pool(name="yw", bufs=4) as yw_pool,
            tc.tile_pool(name="psA", bufs=2, space="PSUM") as psA,
            tc.tile_pool(name="psB", bufs=2, space="PSUM") as psB,
            tc.tile_pool(name="psD", bufs=2, space="PSUM") as psD,
            tc.tile_pool(name="psS", bufs=1, space="PSUM") as psS,
            tc.tile_pool(name="dram", bufs=1, space="DRAM") as dram,
        ):
            # ---- constants
            ident = wpool.tile([P, P], F32, tag="ident")
            make_identity(nc, ident[:])

            # ---- resident weights (bf16), loaded via f32 transient + cast
            w1b = wpool.tile([P, KT * I], BF16, tag="w1b")
            w3b = wpool.tile([P, KT * I], BF16, tag="w3b")
            w2b = wpool.tile([P, IT * H], BF16, tag="w2b")
            wgs = wpool.tile([P, KT * E], F32, tag="wgs")

            for kt in range(KT):
                nc.sync.dma_start(
                    out=wgs[:, kt * E:(kt + 1) * E],
                    in_=wgT_d[kt * P:(kt + 1) * P, :],
                )
            for kt in range(KT):
                wf = wload.tile([P, I], F32, tag="wf")
                nc.sync.dma_start(out=wf[:], in_=w1T_d[kt * P:(kt + 1) * P, :])
                nc.scalar.activation(
                    out=w1b[:, kt * I:(kt + 1) * I], in_=wf[:], func=AF.Copy
                )
            for kt in range(KT):
                wf = wload.tile([P, I], F32, tag="wf")
                nc.sync.dma_start(out=wf[:], in_=w3T_d[kt * P:(kt + 1) * P, :])
                nc.scalar.activation(
                    out=w3b[:, kt * I:(kt + 1) * I], in_=wf[:], func=AF.Copy
                )
            for it in range(IT):
                wf2 = wload.tile([P, H], F32, tag="wf")
                nc.sync.dma_start(out=wf2[:], in_=w2T_d[it * P:(it + 1) * P, :])
                nc.scalar.activation(
                    out=w2b[:, it * H:(it + 1) * H], in_=wf2[:], func=AF.Copy
                )

            # ---- per-RS-block DRAM partials and RS outputs
            partials = [
                dram.tile([H, RS_TOK], BF16, tag=f"part{r}", name=f"part{r}")
                for r in range(NRS)
            ]
            rs_outs = [
                dram.tile([P, RS_TOK], BF16, tag=f"rsout{r}", name=f"rsout{r}")
                for r in range(NRS)
            ]

            for q in range(NCHUNK):
                r = q // CPR          # RS block
                qc = q % CPR          # chunk within RS block
                tok0 = q * CHUNK

                # ---- load xT chunk (f32) and cast to bf16
                xf = xf_pool.tile([P, KT * CHUNK], F32, tag="xf")
                for kt in range(KT):
                    nc.sync.dma_start(
                        out=xf[:, kt * CHUNK:(kt + 1) * CHUNK],
                        in_=xT_d[kt * P:(kt + 1) * P, tok0:tok0 + CHUNK],
                    )
                xb = xb_pool.tile([P, KT * CHUNK], BF16, tag="xb")
                for kt in range(KT):
                    nc.scalar.activation(
                        out=xb[:, kt * CHUNK:(kt + 1) * CHUNK],
                        in_=xf[:, kt * CHUNK:(kt + 1) * CHUNK],
                        func=AF.Copy,
                    )

                # ---- router for this chunk -> combine-weight row [1, CHUNK]
                lch = small.tile([P, TT, E], F32, tag="lch")
                for tt in range(TT):
                    pl = psS.tile([P, E], F32, tag="pl")
                    for kt in range(KT):
                        nc.tensor.matmul(
                            out=pl[:],
                            lhsT=xf[:, kt * CHUNK + tt * P: kt * CHUNK + (tt + 1) * P],
                            rhs=wgs[:, kt * E:(kt + 1) * E],
                            start=(kt == 0),
                            stop=(kt == KT - 1),
                        )
                    nc.vector.tensor_copy(out=lch[:, tt, :], in_=pl[:])

                m1 = small.tile([P, TT], F32, tag="m1")
                nc.vector.reduce_max(out=m1[:], in_=lch[:], axis=AX.X)
                eq1 = small.tile([P, TT, E], F32, tag="eq1")
                nc.vector.tensor_tensor(
                    out=eq1[:], in0=lch[:],
                    in1=m1[:, :, None].broadcast_to([P, TT, E]),
                    op=ALU.is_equal,
                )
                lmask = small.tile([P, TT, E], F32, tag="lmask")
                nc.vector.tensor_scalar(
                    out=lmask[:], in0=eq1[:], scalar1=-1e30, scalar2=None,
                    op0=ALU.mult,
                )
                nc.vector.tensor_tensor(
                    out=lmask[:], in0=lmask[:], in1=lch[:], op=ALU.add
                )
                m2 = small.tile([P, TT], F32, tag="m2")
                nc.vector.reduce_max(out=m2[:], in_=lmask[:], axis=AX.X)
                eq2 = small.tile([P, TT, E], F32, tag="eq2")
                nc.vector.tensor_tensor(
                    out=eq2[:], in0=lmask[:],
                    in1=m2[:, :, None].broadcast_to([P, TT, E]),
                    op=ALU.is_equal,
                )
                d21 = small.tile([P, TT], F32, tag="d21")
                nc.vector.tensor_tensor(out=d21[:], in0=m2[:], in1=m1[:], op=ALU.subtract)
                e2 = small.tile([P, TT], F32, tag="e2")
                nc.scalar.activation(out=e2[:], in_=d21[:], func=AF.Exp)
                den = small.tile([P, TT], F32, tag="den")
                nc.vector.tensor_scalar_add(out=den[:], in0=e2[:], scalar1=1.0)
                inv = small.tile([P, TT], F32, tag="inv")
                nc.vector.reciprocal(out=inv[:], in_=den[:])
                wtop2 = small.tile([P, TT], F32, tag="wtop2")
                nc.vector.tensor_tensor(out=wtop2[:], in0=e2[:], in1=inv[:], op=ALU.mult)

                # per-token weight for OUR expert (core id = expert id).
                # Select column via partition-id–independent trick: each core
                # runs the same program but with its own expert's FFN weights;
                # the gate column is selected by the per-core input `wgT`
                # ordering (see host side: expert c's column is rotated to 0).
                wc = small.tile([P, TT], F32, tag="wc")
                a1 = small.tile([P, TT], F32, tag="a1")
                nc.vector.tensor_tensor(
                    out=a1[:], in0=eq1[:, :, 0], in1=inv[:], op=ALU.mult
                )
                nc.vector.tensor_tensor(
                    out=wc[:], in0=eq2[:, :, 0], in1=wtop2[:], op=ALU.mult
                )
                nc.vector.tensor_tensor(out=wc[:], in0=wc[:], in1=a1[:], op=ALU.add)

                # transpose wc -> row in DRAM -> broadcast to all partitions
                pt = psS.tile([P, P], F32, tag="pt")
                nc.tensor.transpose(out=pt[:TT, :], in_=wc[:], identity=ident[:])
                wcT = small.tile([TT, P], F32, tag="wcT")
                nc.vector.tensor_copy(out=wcT[:], in_=pt[:TT, :])
                wrow_d = dram.tile([1, CHUNK], F32, tag="wrow_d", name="wrow_d",
                                   bufs=2)
                nc.sync.dma_start(
                    out=wrow_d[0, :].rearrange("(p f) -> p f", p=TT), in_=wcT[:]
                )
                wbc = small.tile([P, CHUNK], F32, tag="wbc")
                nc.sync.dma_start(
                    out=wbc[:], in_=wrow_d[:].to_broadcast([P, CHUNK])
                )

                if debug and q == 0:
                    nc.sync.dma_start(out=dbg_lch[:], in_=lch[:].rearrange("p t e -> p (t e)"))
                    nc.sync.dma_start(out=dbg_wc[:], in_=wc[:])
                    nc.sync.dma_start(out=dbg_wbc[:], in_=wbc[:])

                # ---- FFN: h1 = silu(x w1), h3 = x w3, z = h1*h3
                zt = z_pool.tile([P, IT * CHUNK], BF16, tag="zt")
                for it in range(IT):
                    p1 = psA.tile([P, CHUNK], F32, tag="p1")
                    p3 = psB.tile([P, CHUNK], F32, tag="p3")
                    for kt in range(KT):
                        nc.tensor.matmul(
                            out=p1[:],
                            lhsT=w1b[:, kt * I + it * P: kt * I + (it + 1) * P],
                            rhs=xb[:, kt * CHUNK:(kt + 1) * CHUNK],
                            start=(kt == 0),
                            stop=(kt == KT - 1),
                        )
                    for kt in range(KT):
                        nc.tensor.matmul(
                            out=p3[:],
                            lhsT=w3b[:, kt * I + it * P: kt * I + (it + 1) * P],
                            rhs=xb[:, kt * CHUNK:(kt + 1) * CHUNK],
                            start=(kt == 0),
                            stop=(kt == KT - 1),
                        )
                    h1s = small.tile([P, CHUNK], BF16, tag="h1s")
                    nc.scalar.activation(out=h1s[:], in_=p1[:], func=AF.Silu)
                    nc.vector.tensor_tensor(
                        out=zt[:, it * CHUNK:(it + 1) * CHUNK],
                        in0=h1s[:], in1=p3[:], op=ALU.mult,
                    )

                if debug and q == 0:
                    zf0 = small.tile([P, CHUNK], F32, tag="zf0")
                    nc.scalar.activation(out=zf0[:], in_=zt[:, 0:CHUNK], func=AF.Copy)
                    nc.sync.dma_start(out=dbg_z[:], in_=zf0[:])

                # ---- down proj + combine weight, scatter to partial
                for ht in range(KT):
                    pd = psD.tile([P, CHUNK], F32, tag="pd")
                    for it in range(IT):
                        nc.tensor.matmul(
                            out=pd[:],
                            lhsT=w2b[:, it * H + ht * P: it * H + (ht + 1) * P],
                            rhs=zt[:, it * CHUNK:(it + 1) * CHUNK],
                            start=(it == 0),
                            stop=(it == IT - 1),
                        )
                    ywt = yw_pool.tile([P, CHUNK], BF16, tag="ywt")
                    nc.vector.tensor_tensor(
                        out=ywt[:], in0=pd[:], in1=wbc[:], op=ALU.mult,
                    )
                    nc.sync.dma_start(
                        out=partials[r][ht * P:(ht + 1) * P,
                                        qc * CHUNK:(qc + 1) * CHUNK],
                        in_=ywt[:],
                    )

                if debug and q == CPR - 1:
                    for ht2 in range(KT):
                        pbf = wload.tile([P, RS_TOK], BF16, tag="pbf")
                        nc.sync.dma_start(out=pbf[:], in_=partials[0][ht2 * P:(ht2 + 1) * P, :])
                        pff = wload.tile([P, RS_TOK], F32, tag="pff")
                        nc.scalar.activation(out=pff[:], in_=pbf[:], func=AF.Copy)
                        nc.sync.dma_start(out=dbg_part[ht2 * P:(ht2 + 1) * P, :], in_=pff[:])

                # ---- ReduceScatter when RS block complete
                if qc == CPR - 1:
                    nc.gpsimd.collective_compute(
                        "ReduceScatter",
                        ALU.add,
                        replica_groups=[list(range(NCORES))],
                        ins=[partials[r].opt()],
                        outs=[rs_outs[r].opt()],
                    )
                    rsb = wload.tile([P, RS_TOK], BF16, tag="rsb")
                    nc.sync.dma_start(out=rsb[:], in_=rs_outs[r][:])
                    rsf = wload.tile([P, RS_TOK], F32, tag="rsf")
                    nc.scalar.activation(out=rsf[:], in_=rsb[:], func=AF.Copy)
                    nc.sync.dma_start(out=out_d[r], in_=rsf[:])

    _hoist_extra_waits(nc)
    return nc


_NC_CACHE = None


def _get_nc():
    global _NC_CACHE
    if _NC_CACHE is None:
        _NC_CACHE = build_nc()
    return _NC_CACHE


def kernel(hidden_states, wg, w1, w3, w2):
    x = np.asarray(hidden_states, np.float32).reshape(T, H)
    wg = np.asarray(wg, np.float32)
    w1 = np.asarray(w1, np.float32)
    w3 = np.asarray(w3, np.float32)
    w2 = np.asarray(w2, np.float32)

    xT = np.ascontiguousarray(x.T)                      # [H, T]
    in_maps = []
    for c in range(NCORES):
        # rotate gate columns so that this core's expert is column 0
        perm = [(c + k) % E for k in range(E)]
        wgT = np.ascontiguousarray(wg[perm].T)          # [H, E]
        in_maps.append({
            "xT": xT,
            "wgT": wgT,
            "w1T": np.ascontiguousarray(w1[c].T),       # [H, I]
            "w3T": np.ascontiguousarray(w3[c].T),       # [H, I]
            "w2T": np.ascontiguousarray(w2[c].T),       # [I, H]
        })

    res = run_bass_kernel_spmd(_get_nc(), in_maps, list(range(NCORES)))

    outT = np.empty((H, T), np.float32)
    for c in range(NCORES):
        o = res.results[c]["out"]                       # [NRS, P, RS_TOK]
        for r in range(NRS):
            outT[c * P:(c + 1) * P, r * RS_TOK:(r + 1) * RS_TOK] = o[r]
    return np.ascontiguousarray(outT.T).reshape(1, T, H)
